# revision 31
# baseline (speedup 1.0000x reference)
"""Trainium2 Bass kernel for nn_Prior_SPDE (batched SPDE precision matrix assembly).

Problem (hardcoded shapes): B=2, N=1024 (32x32 grid), T=5.
Per (b,t) unit:
    A  = k2*I + m1*Dx + m2*Dy - (H11*Dxx + (H12+H21)*Dxy + H22*Dyy)  [row scaling]
    M  = I + A @ A
    S  = diag(d) @ M          (d = 1/tau^2)
    G  = M^T diag(d) M = S^T @ M   (symmetric)
    Q[t,t] = G_t (+ d_{t+1} I),  Q[t-1,t] = -S_t,  Q[t,t-1] = -S_t^T

A has 9 nonzero diagonals (offsets 0,+-1,+-31,+-32,+-33) => block-tridiagonal in
128-tiles. B=A@A is block-pentadiagonal; G has a 9-tile band (upper half computed,
mirrored on host). Host does the cheap banded assembly of A/A^T and the final Q
placement; the device does all GEMMs via banded 128-tile matmuls in PSUM.

10 units are distributed over 8 cores x 2 slots (cores 0,1 get two real units).
"""

import numpy as np

# ---------------------------------------------------------------- constants
B, N, T = 2, 1024, 5
NX = NY = 32
P = 128
NT = N // P           # 8 row tiles
SLOTS = 2
NCORES = 8
USE_FP32R = True      # PE relaxed-precision fp32 mode (4x matmul throughput)

# unit u = b*T + t; core assignment
UNIT_OF = {}          # (core, slot) -> unit
for u in range(B * T):
    if u < NCORES:
        UNIT_OF[(u, 0)] = u
    else:
        UNIT_OF[(u - NCORES, 1)] = u


def _diff_ops():
    def d1(n, h):
        D = np.zeros((n, n), dtype=np.float32)
        idx = np.arange(n - 1)
        D[idx, idx + 1] = 1.0 / (2.0 * h)
        D[idx + 1, idx] = -1.0 / (2.0 * h)
        return D

    def d2(n, h):
        D = (-2.0 * np.eye(n, dtype=np.float32)
             + np.eye(n, k=1, dtype=np.float32)
             + np.eye(n, k=-1, dtype=np.float32))
        return D / (h * h)

    Ix = np.eye(NX, dtype=np.float32)
    Iy = np.eye(NY, dtype=np.float32)
    Dx = np.kron(Iy, d1(NX, 1.0))
    Dy = np.kron(d1(NY, 1.0), Ix)
    Dxx = np.kron(Iy, d2(NX, 1.0))
    Dyy = np.kron(d2(NY, 1.0), Ix)
    Dxy = Dx @ Dy
    return Dx, Dy, Dxx, Dyy, Dxy


def _band_of(Dm):
    """[NT, P, 3P] band rows: out[k,p,:] = Dm[128k+p, 128(k-1):128(k+2)] (zero padded)."""
    out = np.zeros((NT, P, 3 * P), dtype=np.float32)
    for k in range(NT):
        jlo = max(0, (k - 1) * P)
        jhi = min(N, (k + 2) * P)
        c0 = jlo - (k - 1) * P
        out[k, :, c0:c0 + (jhi - jlo)] = Dm[k * P:(k + 1) * P, jlo:jhi]
    return out


_BANDS = None  # lazy: [6, NT, P, 3P] for I, Dx, Dy, Dxx, Dxy, Dyy


def _get_bands():
    global _BANDS
    if _BANDS is None:
        Dx, Dy, Dxx, Dyy, Dxy = _diff_ops()
        I = np.eye(N, dtype=np.float32)
        _BANDS = np.stack([_band_of(Mm) for Mm in (I, Dx, Dy, Dxx, Dxy, Dyy)])
    return _BANDS


def _coeffs(kappa, m, H, tau):
    """[B,T,6,N] coefficient vectors (signs folded) + d [B,T,N]."""
    tpo = lambda v: np.transpose(v, (0, 2, 1)).astype(np.float32)
    k2 = tpo(kappa[:, 0] ** 2)
    m1, m2 = tpo(m[:, 0]), tpo(m[:, 1])
    H11, H12 = tpo(H[:, 0, 0]), tpo(H[:, 0, 1])
    H21, H22 = tpo(H[:, 1, 0]), tpo(H[:, 1, 1])
    V = np.stack([k2, m1, m2, -H11, -(H12 + H21), -H22], axis=2)  # [B,T,6,N]
    d = (1.0 / tpo(tau[:, 0]) ** 2)
    return V, d


def _build_unit_operands(Vu, du):
    """For one unit: Vu [6,N], du [N] ->
    acomp [P,NT,3P] (A row-band, partition-major),
    atcomp [P,NT,3P] (A^T row-band),
    dvec [P,NT]."""
    bands = _get_bands()
    Vr = Vu.reshape(6, NT, P)                       # [6,k,p]
    # a_knt[k,p,:] = sum_c V[c,k,p] * bands[c,k,p,:]
    a_knt = np.einsum('ckp,ckpj->kpj', Vr, bands)   # [NT,P,3P]
    # A^T tiles: atcomp[k,:, (o+1)P:(o+2)P] = A(i=k+o, k)^T = a_knt[k+o,:, (-o+1)P..].T
    at_knt = np.zeros_like(a_knt)
    for o in (-1, 0, 1):
        ks = [k for k in range(NT) if 0 <= k + o < NT]
        for k in ks:
            tile = a_knt[k + o, :, (-o + 1) * P:(-o + 2) * P]
            at_knt[k, :, (o + 1) * P:(o + 2) * P] = tile.T
    ops = np.ascontiguousarray(a_knt.transpose(1, 0, 2))  # [P,NT,3P]
    dvec = np.ascontiguousarray(du.reshape(NT, P).T)       # [P,NT]
    return ops, dvec


# ---------------------------------------------------------------- device program
_NC = None


def _build_nc(legalize=True, mode='pipe2', iadd='pe', dma_split=2):
    import concourse.bass as bass
    import concourse.mybir as mybir
    import concourse.tile as tile

    nc = bass.Bass("TRN2", target_bir_lowering=True, debug=False)

    opdt = mybir.dt.float32r if USE_FP32R else mybir.dt.float32
    # compact operands: A row-band (j in k-1..k+1); A^T built on device
    ops_d = nc.dram_tensor("ops", [SLOTS, P, NT, 3 * P], opdt,
                           kind="ExternalInput")
    ident_d = nc.dram_tensor("ident", [P, P], opdt, kind="ExternalInput")
    dvec_d = nc.dram_tensor("dvec", [P, SLOTS, NT], mybir.dt.float32,
                            kind="ExternalInput")
    gout_d = nc.dram_tensor("gout", [SLOTS, P, NT, 5 * P], mybir.dt.float32,
                            kind="ExternalOutput")
    sout_d = nc.dram_tensor("sout", [SLOTS, P, NT, 5 * P], mybir.dt.float32,
                            kind="ExternalOutput")

    f32 = mybir.dt.float32

    with tile.TileContext(nc) as tc:
        with (
            tc.tile_pool(name="const", bufs=1) as constp,
            tc.tile_pool(name="a", bufs=2) as apool,
            tc.tile_pool(name="m", bufs=1) as mpool,
            tc.tile_pool(name="s", bufs=2) as spool,
            tc.tile_pool(name="g", bufs=2) as gpool,
            tc.tile_pool(name="ps", bufs=3, space="PSUM") as pspool,
            tc.tile_pool(name="pt", bufs=2, space="PSUM") as tpool,
        ):
            dvec = constp.tile([P, SLOTS, NT], f32)
            nc.sync.dma_start(dvec[:], dvec_d[:])

            # ops storage: cols [0:2P) left pad | [2P:5P) A | [5P:6P) right pad
            #              | [6P:9P) A^T ; identity tile parked at row0 [0:P)
            ops_t = [apool.tile([P, NT, 9 * P], opdt, tag="a", name=f"ops{i}")
                     for i in range(2)]
            m_t = [mpool.tile([P, NT, 5 * P], opdt, tag="m", name=f"m{i}")
                   for i in range(1)]
            s_t = [spool.tile([P, NT, 5 * P], opdt, tag="s", name=f"s{i}")
                   for i in range(2)]
            g_t = [gpool.tile([P, NT, 5 * P], f32, tag="g", name=f"g{i}")
                   for i in range(2)]
            for x in ops_t:
                nc.gpsimd.memset(x[:, :, 0:2 * P].bitcast(f32), 0.0)
                nc.gpsimd.memset(x[:, :, 5 * P:6 * P].bitcast(f32), 0.0)

            step = NT // dma_split
            for s in range(SLOTS):
                ops_sb = ops_t[s % len(ops_t)]
                for h in range(dma_split):
                    nc.sync.dma_start(
                        ops_sb[:, step * h:step * (h + 1), 2 * P:5 * P],
                        ops_d[s, :, step * h:step * (h + 1)])
                nc.sync.dma_start(ops_sb[:, 0, 0:P], ident_d[:])

            for s in range(SLOTS):
                ops_sb = ops_t[s % len(ops_t)]
                m_sb = m_t[s % len(m_t)]
                s_sb = s_t[s % len(s_t)]
                g_sb = g_t[s % len(g_t)]
                itile = ops_sb[:, 0, 0:P]

                # A^T tiles via PE transpose: AT(k,i) = A(i,k)^T, stored at
                # ops_sb[:, k, (6+i-k+1)P : (6+i-k+2)P]. B row i needs only
                # row i's own transposes, so T(i) is emitted just before B(i);
                # G(i) needs B rows <= i+2, so it is interleaved after B(i+2).
                def emit_T(i):
                    for k in (i - 1, i, i + 1):
                        if not 0 <= k < NT:
                            continue
                        tp = tpool.tile([P, P], opdt, tag="pt", name="tp")
                        nc.tensor.transpose(
                            tp[:], ops_sb[:, i, (k - i + 3) * P:(k - i + 4) * P],
                            itile)
                        nc.scalar.copy(
                            ops_sb[:, k, (6 + i - k + 1) * P:(6 + i - k + 2) * P],
                            tp[:])

                def emit_B(i):
                    pb = pspool.tile([P, 6 * P], f32, tag="ps", name="pb")
                    ks = [k for k in (i - 1, i, i + 1) if 0 <= k < NT]
                    for idx, k in enumerate(ks):
                        c = (i - k + 1) * P
                        nc.tensor.matmul(
                            pb[:, 0:4 * P],
                            ops_sb[:, k, 6 * P + c:6 * P + c + P],
                            ops_sb[:, k, c:c + 4 * P],
                            start=(idx == 0),
                            stop=(idx == len(ks) - 1 and iadd != 'pe'))
                    if iadd == 'pe':
                        nc.tensor.matmul(pb[:, 2 * P:3 * P], itile, itile,
                                         start=False, stop=True)
                    if i + 2 < NT:
                        k = i + 1
                        nc.tensor.matmul(
                            pb[:, 4 * P:6 * P],
                            ops_sb[:, k, (6 + i - k + 1) * P:(6 + i - k + 2) * P],
                            ops_sb[:, k, 4 * P:6 * P],
                            start=True, stop=True)
                    jlo, jhi = max(0, i - 2), min(NT - 1, i + 2)
                    clo, chi = (jlo - i + 2) * P, (jhi - i + 3) * P
                    if iadd == 'pe':
                        nc.vector.tensor_copy(m_sb[:, i, clo:chi],
                                              pb[:, clo:chi])
                    else:
                        if clo < 2 * P:
                            nc.vector.tensor_copy(m_sb[:, i, clo:2 * P],
                                                  pb[:, clo:2 * P])
                        nc.vector.tensor_tensor(m_sb[:, i, 2 * P:3 * P],
                                                pb[:, 2 * P:3 * P], itile,
                                                mybir.AluOpType.add)
                        if chi > 3 * P:
                            nc.vector.tensor_copy(m_sb[:, i, 3 * P:chi],
                                                  pb[:, 3 * P:chi])
                    nc.vector.tensor_scalar_mul(s_sb[:, i, clo:chi],
                                                m_sb[:, i, clo:chi],
                                                dvec[:, s, i:i + 1])
                    nc.sync.dma_start(sout_d[s, :, i, clo:chi],
                                      s_sb[:, i, clo:chi].bitcast(f32))

                def emit_G(i):
                    pg = pspool.tile([P, 6 * P], f32, tag="ps", name="pg")
                    ks = [k for k in range(i + 2, i - 3, -1) if 0 <= k < NT]
                    for idx, k in enumerate(ks):
                        c = (i - k + 2) * P
                        w = (min(i + 3, k + 2, NT - 1) - i + 1) * P
                        nc.tensor.matmul(
                            pg[:, 0:w],
                            s_sb[:, k, c:c + P],
                            m_sb[:, k, c:c + w],
                            start=(idx == 0), stop=(idx == len(ks) - 1))
                    if i + 4 < NT:
                        k = i + 2
                        nc.tensor.matmul(
                            pg[:, 4 * P:5 * P],
                            s_sb[:, k, 0:P],
                            m_sb[:, k, 4 * P:5 * P],
                            start=True, stop=True)
                    w = (min(NT - 1, i + 4) - i + 1) * P
                    nc.scalar.copy(g_sb[:, i, 0:w], pg[:, 0:w])
                    nc.sync.dma_start(gout_d[s, :, i, 0:w], g_sb[:, i, 0:w])

                if mode == 'interleave':
                    for i in range(NT):
                        emit_T(i)
                        emit_B(i)
                        if i >= 2:
                            emit_G(i - 2)
                    for i in range(NT - 2, NT):
                        emit_G(i)
                elif mode == 'phases':
                    for i in range(NT):
                        emit_T(i)
                    for i in range(NT):
                        emit_B(i)
                    for i in range(NT):
                        emit_G(i)
                elif mode == 'tb_g':
                    for i in range(NT):
                        emit_T(i)
                        emit_B(i)
                    for i in range(NT):
                        emit_G(i)
                elif mode == 'pipe2':
                    emit_T(0); emit_T(1)
                    for i in range(NT):
                        if i + 2 < NT:
                            emit_T(i + 2)
                        emit_B(i)
                    for i in range(NT):
                        emit_G(i)

    if legalize:
        _legalize_waits(nc)
    return nc


def _legalize_waits(nc):
    """This toolchain's walrus codegen accepts at most ONE sync-wait per
    engine instruction; Tile emits more. Two legal rewrites:
    - drop same-engine self-waits whose value is already provided by
      same-engine instructions earlier in block order (engines issue and
      complete their stream in order, so FIFO order subsumes the wait);
    - split remaining multi-wait instructions: prepend same-engine Drain
      instructions carrying the extra waits one each.
    Call only for the HW build; CoreSim's race detector does not credit
    engine-FIFO ordering and would flag the dropped waits."""
    import concourse.mybir as mybir
    import bass_rust as _br

    nid = 0
    for blk in nc.m.functions[0].blocks:
        out = []
        upd = {}
        for ins in blk.instructions:
            si = ins.sync_info
            if si is None:
                out.append(ins)
                continue
            waits = list(si.on_wait)
            if len(waits) > 1:
                eng = str(ins.engine)
                kept = []
                for w in waits:
                    owner = upd.get(w.ant_name)
                    ticks = upd.get((w.ant_name, "n"), 0)
                    if owner == eng and w.wait_mode == "sem-ge-imm" \
                            and ticks >= (w.wait_value or 0):
                        continue  # provably satisfied by engine FIFO order
                    kept.append(w)
                waits = kept
            if len(waits) > 1:
                for w in waits[:-1]:
                    d = mybir.InstDrain(name=f"I-waitsplit-{nid}")
                    nid += 1
                    d.engine = ins.engine
                    d.sync_info = _br.SyncInfo(on_wait=[w], on_update=[])
                    out.append(d)
                waits = waits[-1:]
            if len(waits) != len(si.on_wait):
                ins.sync_info = _br.SyncInfo(on_wait=waits,
                                             on_update=list(si.on_update))
            out.append(ins)
            for u in si.on_update:
                if u.update_mode == "sem-inc":
                    upd[u.ant_name] = str(ins.engine)
                    upd[(u.ant_name, "n")] = upd.get((u.ant_name, "n"), 0) \
                        + (u.update_value or 0)
        blk.instructions = out


def _get_nc():
    global _NC
    if _NC is None:
        _NC = _build_nc()
    return _NC


# ---------------------------------------------------------------- host wrapper
def _make_in_maps(kappa, m, H, tau):
    V, d = _coeffs(kappa, m, H, tau)
    zero_a = np.zeros((P, NT, 3 * P), dtype=np.float32)
    zero_d = np.zeros((P, NT), dtype=np.float32)
    ident = np.eye(P, dtype=np.float32)
    in_maps = []
    for c in range(NCORES):
        ac, dv = [], []
        for s in range(SLOTS):
            u = UNIT_OF.get((c, s))
            if u is None:
                ac.append(zero_a); dv.append(zero_d)
            else:
                b, t = divmod(u, T)
                a1, d1 = _build_unit_operands(V[b, t], d[b, t])
                ac.append(a1); dv.append(d1)
        in_maps.append({
            "ops": np.stack(ac),                                     # [S,P,NT,6P]
            "dvec": np.ascontiguousarray(np.stack(dv, axis=1)),      # [P,S,NT]
            "ident": ident,
        })
    return in_maps, d


def _assemble_Q(results, d):
    Q = np.zeros((B, T * N, T * N), dtype=np.float32)
    for (c, s), u in UNIT_OF.items():
        b, t = divmod(u, T)
        gout = results[c]["gout"][s]    # [P, NT, 5P]
        sout = results[c]["sout"][s]
        r0 = t * N
        Qb = Q[b]
        # diagonal block: G (+ mirror)
        for i in range(NT):
            for dj in range(0, min(5, NT - i)):
                j = i + dj
                tile = gout[:, i, dj * P:(dj + 1) * P]
                Qb[r0 + i * P:r0 + (i + 1) * P, r0 + j * P:r0 + (j + 1) * P] = tile
                if dj:
                    Qb[r0 + j * P:r0 + (j + 1) * P, r0 + i * P:r0 + (i + 1) * P] = tile.T
        if t >= 1:
            rp = (t - 1) * N
            for k in range(NT):
                jlo, jhi = max(0, k - 2), min(NT - 1, k + 2)
                stile = sout[:, k, (jlo - k + 2) * P:(jhi - k + 3) * P]
                Qb[rp + k * P:rp + (k + 1) * P, r0 + jlo * P:r0 + (jhi + 1) * P] = -stile
                Qb[r0 + jlo * P:r0 + (jhi + 1) * P, rp + k * P:rp + (k + 1) * P] = -stile.T
    idx = np.arange(T * N)
    for b in range(B):
        for t in range(T - 1):
            r = idx[t * N:(t + 1) * N]
            Q[b, r, r] += d[b, t + 1]
    return Q




_RUNNER = None


def _get_runner():
    """Cached jitted shard_map executable over the 8 cores (no donation —
    host reads only valid output regions, so pre-zeroed outputs are not
    required). Mirrors concourse.bass2jax.run_bass_via_pjrt."""
    global _RUNNER
    if _RUNNER is not None:
        return _RUNNER
    import jax
    import numpy as np
    import concourse.mybir as mybir
    from jax.sharding import Mesh, PartitionSpec
    from jax.experimental.shard_map import shard_map
    from concourse import bass2jax

    bass2jax.install_neuronx_cc_hook()
    nc = _get_nc()

    pname = nc.partition_id_tensor.name if nc.partition_id_tensor else None
    in_names, out_names, out_avals, zero_outs = [], [], [], []
    for alloc in nc.m.functions[0].allocations:
        if not isinstance(alloc, mybir.MemoryLocationSet):
            continue
        name = alloc.memorylocations[0].name
        if alloc.kind == "ExternalInput":
            if name != pname:
                in_names.append(name)
        elif alloc.kind == "ExternalOutput":
            out_names.append(name)
            dt = mybir.dt.np(alloc.dtype)
            out_avals.append(jax.core.ShapedArray(tuple(alloc.tensor_shape), dt))
            zero_outs.append(np.zeros(tuple(alloc.tensor_shape), dt))
    n_params = len(in_names)
    all_names = in_names + out_names

    def _body(*args):
        operands = list(args)
        if pname is not None:
            operands.append(bass2jax.partition_id_tensor())
        outs = bass2jax._bass_exec_p.bind(
            *operands,
            out_avals=tuple(out_avals),
            in_names=tuple(all_names + ([pname] if pname else [])),
            out_names=tuple(out_names),
            lowering_input_output_aliases=(),
            sim_require_finite=True,
            sim_require_nnan=True,
            nc=nc,
        )
        return tuple(outs)

    devices = jax.devices()[:NCORES]
    mesh = Mesh(np.asarray(devices), ("core",))
    nin = n_params + len(zero_outs)
    f = jax.jit(shard_map(
        _body, mesh=mesh,
        in_specs=(PartitionSpec("core"),) * nin,
        out_specs=(PartitionSpec("core"),) * len(out_names),
        check_rep=False))

    def run(in_maps):
        cin = [np.concatenate([np.asarray(m[nm]) for m in in_maps], axis=0)
               for nm in in_names]
        czero = [np.zeros((NCORES * z.shape[0], *z.shape[1:]), z.dtype)
                 for z in zero_outs]
        outs = f(*cin, *czero)
        return [
            {nm: np.asarray(outs[i]).reshape(NCORES, *out_avals[i].shape)[c]
             for i, nm in enumerate(out_names)}
            for c in range(NCORES)
        ]

    _RUNNER = (run, f, in_names, zero_outs)
    return _RUNNER


def _run_device(in_maps):
    run, _f, _inn, _z = _get_runner()
    return run(in_maps)


def _run_fake_device(in_maps):
    """Numpy emulation of the device program (same in/out layouts)."""
    out = []
    for im in in_maps:
        gout = np.zeros((SLOTS, P, NT, 5 * P), dtype=np.float32)
        sout = np.zeros((SLOTS, P, NT, 5 * P), dtype=np.float32)
        for s in range(SLOTS):
            acomp = im["ops"][s]                  # [P,NT,3P]
            dv = im["dvec"][:, s]           # [P,NT]
            A = np.zeros((N, N), dtype=np.float32)
            for k in range(NT):
                jlo = max(0, (k - 1) * P)
                jhi = min(N, (k + 2) * P)
                c0 = jlo - (k - 1) * P
                A[k * P:(k + 1) * P, jlo:jhi] = acomp[:, k, c0:c0 + jhi - jlo]
            M = np.eye(N, dtype=np.float32) + A @ A
            dfull = dv.T.reshape(N)
            S = dfull[:, None] * M
            G = S.T @ M
            for i in range(NT):
                jlo, jhi = max(0, i - 2), min(NT - 1, i + 2)
                sout[s, :, i, (jlo - i + 2) * P:(jhi - i + 3) * P] = \
                    S[i * P:(i + 1) * P, jlo * P:(jhi + 1) * P]
                w = min(NT - 1, i + 4) - i + 1
                gout[s, :, i, 0:w * P] = G[i * P:(i + 1) * P, i * P:(i + w) * P]
        out.append({"gout": gout, "sout": sout})
    return out


def kernel(kappa, m, H, tau, n_x=32, _fake=False):
    kappa = np.asarray(kappa, dtype=np.float32)
    m = np.asarray(m, dtype=np.float32)
    H = np.asarray(H, dtype=np.float32)
    tau = np.asarray(tau, dtype=np.float32)
    assert int(n_x) == NX
    in_maps, d = _make_in_maps(kappa, m, H, tau)
    results = _run_fake_device(in_maps) if _fake else _run_device(in_maps)
    return _assemble_Q(results, d)


# revision 33
# speedup vs baseline: 1286.0233x; 1286.0233x over previous
"""Trainium2 Bass kernel for nn_Prior_SPDE (batched SPDE precision matrix assembly).

Problem (hardcoded shapes): B=2, N=1024 (32x32 grid), T=5.
Per (b,t) unit:
    A  = k2*I + m1*Dx + m2*Dy - (H11*Dxx + (H12+H21)*Dxy + H22*Dyy)  [row scaling]
    M  = I + A @ A
    S  = diag(d) @ M          (d = 1/tau^2)
    G  = M^T diag(d) M = S^T @ M   (symmetric)
    Q[t,t] = G_t (+ d_{t+1} I),  Q[t-1,t] = -S_t,  Q[t,t-1] = -S_t^T

A has 9 nonzero diagonals (offsets 0,+-1,+-31,+-32,+-33) => block-tridiagonal in
128-tiles. B=A@A is block-pentadiagonal; G has a 9-tile band (upper half computed,
mirrored on host). Host does the cheap banded assembly of A/A^T and the final Q
placement; the device does all GEMMs via banded 128-tile matmuls in PSUM.

10 units are distributed over 8 cores x 2 slots (cores 0,1 get two real units).
"""

import numpy as np

# ---------------------------------------------------------------- constants
B, N, T = 2, 1024, 5
NX = NY = 32
P = 128
NT = N // P           # 8 row tiles
SLOTS = 2
NCORES = 8
USE_FP32R = True      # PE relaxed-precision fp32 mode (4x matmul throughput)

# unit u = b*T + t; core assignment
UNIT_OF = {}          # (core, slot) -> unit
for u in range(B * T):
    if u < NCORES:
        UNIT_OF[(u, 0)] = u
    else:
        UNIT_OF[(u - NCORES, 1)] = u


def _diff_ops():
    def d1(n, h):
        D = np.zeros((n, n), dtype=np.float32)
        idx = np.arange(n - 1)
        D[idx, idx + 1] = 1.0 / (2.0 * h)
        D[idx + 1, idx] = -1.0 / (2.0 * h)
        return D

    def d2(n, h):
        D = (-2.0 * np.eye(n, dtype=np.float32)
             + np.eye(n, k=1, dtype=np.float32)
             + np.eye(n, k=-1, dtype=np.float32))
        return D / (h * h)

    Ix = np.eye(NX, dtype=np.float32)
    Iy = np.eye(NY, dtype=np.float32)
    Dx = np.kron(Iy, d1(NX, 1.0))
    Dy = np.kron(d1(NY, 1.0), Ix)
    Dxx = np.kron(Iy, d2(NX, 1.0))
    Dyy = np.kron(d2(NY, 1.0), Ix)
    Dxy = Dx @ Dy
    return Dx, Dy, Dxx, Dyy, Dxy


def _band_of(Dm):
    """[NT, P, 3P] band rows: out[k,p,:] = Dm[128k+p, 128(k-1):128(k+2)] (zero padded)."""
    out = np.zeros((NT, P, 3 * P), dtype=np.float32)
    for k in range(NT):
        jlo = max(0, (k - 1) * P)
        jhi = min(N, (k + 2) * P)
        c0 = jlo - (k - 1) * P
        out[k, :, c0:c0 + (jhi - jlo)] = Dm[k * P:(k + 1) * P, jlo:jhi]
    return out


_BANDS = None  # lazy: [6, NT, P, 3P] for I, Dx, Dy, Dxx, Dxy, Dyy


def _get_bands():
    global _BANDS
    if _BANDS is None:
        Dx, Dy, Dxx, Dyy, Dxy = _diff_ops()
        I = np.eye(N, dtype=np.float32)
        _BANDS = np.stack([_band_of(Mm) for Mm in (I, Dx, Dy, Dxx, Dxy, Dyy)])
    return _BANDS


def _coeffs(kappa, m, H, tau):
    """[B,T,6,N] coefficient vectors (signs folded) + d [B,T,N]."""
    tpo = lambda v: np.transpose(v, (0, 2, 1)).astype(np.float32)
    k2 = tpo(kappa[:, 0] ** 2)
    m1, m2 = tpo(m[:, 0]), tpo(m[:, 1])
    H11, H12 = tpo(H[:, 0, 0]), tpo(H[:, 0, 1])
    H21, H22 = tpo(H[:, 1, 0]), tpo(H[:, 1, 1])
    V = np.stack([k2, m1, m2, -H11, -(H12 + H21), -H22], axis=2)  # [B,T,6,N]
    d = (1.0 / tpo(tau[:, 0]) ** 2)
    return V, d


def _build_unit_operands(Vu, du):
    """For one unit: Vu [6,N], du [N] ->
    acomp [P,NT,3P] (A row-band, partition-major),
    atcomp [P,NT,3P] (A^T row-band),
    dvec [P,NT]."""
    bands = _get_bands()
    Vr = Vu.reshape(6, NT, P)                       # [6,k,p]
    # a_knt[k,p,:] = sum_c V[c,k,p] * bands[c,k,p,:]
    a_knt = np.einsum('ckp,ckpj->kpj', Vr, bands)   # [NT,P,3P]
    # A^T tiles: atcomp[k,:, (o+1)P:(o+2)P] = A(i=k+o, k)^T = a_knt[k+o,:, (-o+1)P..].T
    at_knt = np.zeros_like(a_knt)
    for o in (-1, 0, 1):
        ks = [k for k in range(NT) if 0 <= k + o < NT]
        for k in ks:
            tile = a_knt[k + o, :, (-o + 1) * P:(-o + 2) * P]
            at_knt[k, :, (o + 1) * P:(o + 2) * P] = tile.T
    ops = np.ascontiguousarray(a_knt.transpose(1, 0, 2))  # [P,NT,3P]
    dvec = np.ascontiguousarray(du.reshape(NT, P).T)       # [P,NT]
    return ops, dvec


# ---------------------------------------------------------------- device program
_NC = None


def _build_nc(legalize=True, mode='pipe2', iadd='pe', dma_split=2, psb=2, ptb=4):
    import concourse.bass as bass
    import concourse.mybir as mybir
    import concourse.tile as tile

    nc = bass.Bass("TRN2", target_bir_lowering=True, debug=False)

    opdt = mybir.dt.float32r if USE_FP32R else mybir.dt.float32
    # compact operands: A row-band (j in k-1..k+1); A^T built on device
    ops_d = nc.dram_tensor("ops", [SLOTS, P, NT, 3 * P], opdt,
                           kind="ExternalInput")
    ident_d = nc.dram_tensor("ident", [P, P], opdt, kind="ExternalInput")
    dvec_d = nc.dram_tensor("dvec", [P, SLOTS, NT], mybir.dt.float32,
                            kind="ExternalInput")
    gout_d = nc.dram_tensor("gout", [SLOTS, P, NT, 5 * P], mybir.dt.float32,
                            kind="ExternalOutput")
    sout_d = nc.dram_tensor("sout", [SLOTS, P, NT, 5 * P], mybir.dt.float32,
                            kind="ExternalOutput")

    f32 = mybir.dt.float32

    with tile.TileContext(nc) as tc:
        with (
            tc.tile_pool(name="const", bufs=1) as constp,
            tc.tile_pool(name="a", bufs=2) as apool,
            tc.tile_pool(name="m", bufs=1) as mpool,
            tc.tile_pool(name="s", bufs=2) as spool,
            tc.tile_pool(name="g", bufs=2) as gpool,
            tc.tile_pool(name="ps", bufs=psb, space="PSUM") as pspool,
            tc.tile_pool(name="pt", bufs=ptb, space="PSUM") as tpool,
        ):
            dvec = constp.tile([P, SLOTS, NT], f32)
            nc.sync.dma_start(dvec[:], dvec_d[:])

            # ops storage: cols [0:2P) left pad | [2P:5P) A | [5P:6P) right pad
            #              | [6P:9P) A^T ; identity tile parked at row0 [0:P)
            ops_t = [apool.tile([P, NT, 9 * P], opdt, tag="a", name=f"ops{i}")
                     for i in range(2)]
            m_t = [mpool.tile([P, NT, 5 * P], opdt, tag="m", name=f"m{i}")
                   for i in range(1)]
            s_t = [spool.tile([P, NT, 5 * P], opdt, tag="s", name=f"s{i}")
                   for i in range(2)]
            g_t = [gpool.tile([P, NT, 5 * P], f32, tag="g", name=f"g{i}")
                   for i in range(2)]
            for x in ops_t:
                nc.gpsimd.memset(x[:, :, 0:2 * P].bitcast(f32), 0.0)
                nc.gpsimd.memset(x[:, :, 5 * P:6 * P].bitcast(f32), 0.0)

            step = NT // dma_split
            for s in range(SLOTS):
                ops_sb = ops_t[s % len(ops_t)]
                for h in range(dma_split):
                    nc.sync.dma_start(
                        ops_sb[:, step * h:step * (h + 1), 2 * P:5 * P],
                        ops_d[s, :, step * h:step * (h + 1)])
                nc.sync.dma_start(ops_sb[:, 0, 0:P], ident_d[:])

            for s in range(SLOTS):
                ops_sb = ops_t[s % len(ops_t)]
                m_sb = m_t[s % len(m_t)]
                s_sb = s_t[s % len(s_t)]
                g_sb = g_t[s % len(g_t)]
                itile = ops_sb[:, 0, 0:P]

                # A^T tiles via PE transpose: AT(k,i) = A(i,k)^T, stored at
                # ops_sb[:, k, (6+i-k+1)P : (6+i-k+2)P]. B row i needs only
                # row i's own transposes, so T(i) is emitted just before B(i);
                # G(i) needs B rows <= i+2, so it is interleaved after B(i+2).
                def emit_T(i):
                    for k in (i - 1, i, i + 1):
                        if not 0 <= k < NT:
                            continue
                        tp = tpool.tile([P, P], opdt, tag="pt", name="tp")
                        nc.tensor.transpose(
                            tp[:], ops_sb[:, i, (k - i + 3) * P:(k - i + 4) * P],
                            itile)
                        nc.scalar.copy(
                            ops_sb[:, k, (6 + i - k + 1) * P:(6 + i - k + 2) * P],
                            tp[:])

                def emit_B(i):
                    pb = pspool.tile([P, 6 * P], f32, tag="ps", name="pb")
                    ks = [k for k in (i - 1, i, i + 1) if 0 <= k < NT]
                    for idx, k in enumerate(ks):
                        c = (i - k + 1) * P
                        nc.tensor.matmul(
                            pb[:, 0:4 * P],
                            ops_sb[:, k, 6 * P + c:6 * P + c + P],
                            ops_sb[:, k, c:c + 4 * P],
                            start=(idx == 0),
                            stop=(idx == len(ks) - 1 and iadd != 'pe'))
                    if iadd == 'pe':
                        nc.tensor.matmul(pb[:, 2 * P:3 * P], itile, itile,
                                         start=False, stop=True)
                    if i + 2 < NT:
                        k = i + 1
                        nc.tensor.matmul(
                            pb[:, 4 * P:6 * P],
                            ops_sb[:, k, (6 + i - k + 1) * P:(6 + i - k + 2) * P],
                            ops_sb[:, k, 4 * P:6 * P],
                            start=True, stop=True)
                    jlo, jhi = max(0, i - 2), min(NT - 1, i + 2)
                    clo, chi = (jlo - i + 2) * P, (jhi - i + 3) * P
                    if iadd == 'pe':
                        nc.vector.tensor_copy(m_sb[:, i, clo:chi],
                                              pb[:, clo:chi])
                    else:
                        if clo < 2 * P:
                            nc.vector.tensor_copy(m_sb[:, i, clo:2 * P],
                                                  pb[:, clo:2 * P])
                        nc.vector.tensor_tensor(m_sb[:, i, 2 * P:3 * P],
                                                pb[:, 2 * P:3 * P], itile,
                                                mybir.AluOpType.add)
                        if chi > 3 * P:
                            nc.vector.tensor_copy(m_sb[:, i, 3 * P:chi],
                                                  pb[:, 3 * P:chi])
                    nc.vector.tensor_scalar_mul(s_sb[:, i, clo:chi],
                                                m_sb[:, i, clo:chi],
                                                dvec[:, s, i:i + 1])
                    nc.sync.dma_start(sout_d[s, :, i, clo:chi],
                                      s_sb[:, i, clo:chi].bitcast(f32))

                def emit_G(i):
                    pg = pspool.tile([P, 6 * P], f32, tag="ps", name="pg")
                    ks = [k for k in range(i + 2, i - 3, -1) if 0 <= k < NT]
                    for idx, k in enumerate(ks):
                        c = (i - k + 2) * P
                        w = (min(i + 3, k + 2, NT - 1) - i + 1) * P
                        nc.tensor.matmul(
                            pg[:, 0:w],
                            s_sb[:, k, c:c + P],
                            m_sb[:, k, c:c + w],
                            start=(idx == 0), stop=(idx == len(ks) - 1))
                    if i + 4 < NT:
                        k = i + 2
                        nc.tensor.matmul(
                            pg[:, 4 * P:5 * P],
                            s_sb[:, k, 0:P],
                            m_sb[:, k, 4 * P:5 * P],
                            start=True, stop=True)
                    w = (min(NT - 1, i + 4) - i + 1) * P
                    nc.scalar.copy(g_sb[:, i, 0:w], pg[:, 0:w])
                    nc.sync.dma_start(gout_d[s, :, i, 0:w], g_sb[:, i, 0:w])

                if mode == 'interleave':
                    for i in range(NT):
                        emit_T(i)
                        emit_B(i)
                        if i >= 2:
                            emit_G(i - 2)
                    for i in range(NT - 2, NT):
                        emit_G(i)
                elif mode == 'phases':
                    for i in range(NT):
                        emit_T(i)
                    for i in range(NT):
                        emit_B(i)
                    for i in range(NT):
                        emit_G(i)
                elif mode == 'tb_g':
                    for i in range(NT):
                        emit_T(i)
                        emit_B(i)
                    for i in range(NT):
                        emit_G(i)
                elif mode == 'pipe2':
                    emit_T(0); emit_T(1)
                    for i in range(NT):
                        if i + 2 < NT:
                            emit_T(i + 2)
                        emit_B(i)
                    for i in range(NT):
                        emit_G(i)

    if legalize:
        _legalize_waits(nc)
    return nc


def _legalize_waits(nc):
    """This toolchain's walrus codegen accepts at most ONE sync-wait per
    engine instruction; Tile emits more. Two legal rewrites:
    - drop same-engine self-waits whose value is already provided by
      same-engine instructions earlier in block order (engines issue and
      complete their stream in order, so FIFO order subsumes the wait);
    - split remaining multi-wait instructions: prepend same-engine Drain
      instructions carrying the extra waits one each.
    Call only for the HW build; CoreSim's race detector does not credit
    engine-FIFO ordering and would flag the dropped waits."""
    import concourse.mybir as mybir
    import bass_rust as _br

    nid = 0
    for blk in nc.m.functions[0].blocks:
        out = []
        upd = {}
        for ins in blk.instructions:
            si = ins.sync_info
            if si is None:
                out.append(ins)
                continue
            waits = list(si.on_wait)
            if len(waits) > 1:
                eng = str(ins.engine)
                kept = []
                for w in waits:
                    owner = upd.get(w.ant_name)
                    ticks = upd.get((w.ant_name, "n"), 0)
                    if owner == eng and w.wait_mode == "sem-ge-imm" \
                            and ticks >= (w.wait_value or 0):
                        continue  # provably satisfied by engine FIFO order
                    kept.append(w)
                waits = kept
            if len(waits) > 1:
                for w in waits[:-1]:
                    d = mybir.InstDrain(name=f"I-waitsplit-{nid}")
                    nid += 1
                    d.engine = ins.engine
                    d.sync_info = _br.SyncInfo(on_wait=[w], on_update=[])
                    out.append(d)
                waits = waits[-1:]
            if len(waits) != len(si.on_wait):
                ins.sync_info = _br.SyncInfo(on_wait=waits,
                                             on_update=list(si.on_update))
            out.append(ins)
            for u in si.on_update:
                if u.update_mode == "sem-inc":
                    upd[u.ant_name] = str(ins.engine)
                    upd[(u.ant_name, "n")] = upd.get((u.ant_name, "n"), 0) \
                        + (u.update_value or 0)
        blk.instructions = out


def _get_nc():
    global _NC
    if _NC is None:
        _NC = _build_nc()
    return _NC


# ---------------------------------------------------------------- host wrapper
def _make_in_maps(kappa, m, H, tau):
    V, d = _coeffs(kappa, m, H, tau)
    zero_a = np.zeros((P, NT, 3 * P), dtype=np.float32)
    zero_d = np.zeros((P, NT), dtype=np.float32)
    ident = np.eye(P, dtype=np.float32)
    in_maps = []
    for c in range(NCORES):
        ac, dv = [], []
        for s in range(SLOTS):
            u = UNIT_OF.get((c, s))
            if u is None:
                ac.append(zero_a); dv.append(zero_d)
            else:
                b, t = divmod(u, T)
                a1, d1 = _build_unit_operands(V[b, t], d[b, t])
                ac.append(a1); dv.append(d1)
        in_maps.append({
            "ops": np.stack(ac),                                     # [S,P,NT,6P]
            "dvec": np.ascontiguousarray(np.stack(dv, axis=1)),      # [P,S,NT]
            "ident": ident,
        })
    return in_maps, d


def _assemble_Q(results, d):
    Q = np.zeros((B, T * N, T * N), dtype=np.float32)
    for (c, s), u in UNIT_OF.items():
        b, t = divmod(u, T)
        gout = results[c]["gout"][s]    # [P, NT, 5P]
        sout = results[c]["sout"][s]
        r0 = t * N
        Qb = Q[b]
        # diagonal block: G (+ mirror)
        for i in range(NT):
            for dj in range(0, min(5, NT - i)):
                j = i + dj
                tile = gout[:, i, dj * P:(dj + 1) * P]
                Qb[r0 + i * P:r0 + (i + 1) * P, r0 + j * P:r0 + (j + 1) * P] = tile
                if dj:
                    Qb[r0 + j * P:r0 + (j + 1) * P, r0 + i * P:r0 + (i + 1) * P] = tile.T
        if t >= 1:
            rp = (t - 1) * N
            for k in range(NT):
                jlo, jhi = max(0, k - 2), min(NT - 1, k + 2)
                stile = sout[:, k, (jlo - k + 2) * P:(jhi - k + 3) * P]
                Qb[rp + k * P:rp + (k + 1) * P, r0 + jlo * P:r0 + (jhi + 1) * P] = -stile
                Qb[r0 + jlo * P:r0 + (jhi + 1) * P, rp + k * P:rp + (k + 1) * P] = -stile.T
    idx = np.arange(T * N)
    for b in range(B):
        for t in range(T - 1):
            r = idx[t * N:(t + 1) * N]
            Q[b, r, r] += d[b, t + 1]
    return Q




_RUNNER = None


def _get_runner():
    """Cached jitted shard_map executable over the 8 cores (no donation —
    host reads only valid output regions, so pre-zeroed outputs are not
    required). Mirrors concourse.bass2jax.run_bass_via_pjrt."""
    global _RUNNER
    if _RUNNER is not None:
        return _RUNNER
    import jax
    import numpy as np
    import concourse.mybir as mybir
    from jax.sharding import Mesh, PartitionSpec
    from jax.experimental.shard_map import shard_map
    from concourse import bass2jax

    bass2jax.install_neuronx_cc_hook()
    nc = _get_nc()

    pname = nc.partition_id_tensor.name if nc.partition_id_tensor else None
    in_names, out_names, out_avals, zero_outs = [], [], [], []
    for alloc in nc.m.functions[0].allocations:
        if not isinstance(alloc, mybir.MemoryLocationSet):
            continue
        name = alloc.memorylocations[0].name
        if alloc.kind == "ExternalInput":
            if name != pname:
                in_names.append(name)
        elif alloc.kind == "ExternalOutput":
            out_names.append(name)
            dt = mybir.dt.np(alloc.dtype)
            out_avals.append(jax.core.ShapedArray(tuple(alloc.tensor_shape), dt))
            zero_outs.append(np.zeros(tuple(alloc.tensor_shape), dt))
    n_params = len(in_names)
    all_names = in_names + out_names

    def _body(*args):
        operands = list(args)
        if pname is not None:
            operands.append(bass2jax.partition_id_tensor())
        outs = bass2jax._bass_exec_p.bind(
            *operands,
            out_avals=tuple(out_avals),
            in_names=tuple(all_names + ([pname] if pname else [])),
            out_names=tuple(out_names),
            lowering_input_output_aliases=(),
            sim_require_finite=True,
            sim_require_nnan=True,
            nc=nc,
        )
        return tuple(outs)

    devices = jax.devices()[:NCORES]
    mesh = Mesh(np.asarray(devices), ("core",))
    nin = n_params + len(zero_outs)
    f = jax.jit(shard_map(
        _body, mesh=mesh,
        in_specs=(PartitionSpec("core"),) * nin,
        out_specs=(PartitionSpec("core"),) * len(out_names),
        check_rep=False))

    def run(in_maps):
        cin = [np.concatenate([np.asarray(m[nm]) for m in in_maps], axis=0)
               for nm in in_names]
        czero = [np.zeros((NCORES * z.shape[0], *z.shape[1:]), z.dtype)
                 for z in zero_outs]
        outs = f(*cin, *czero)
        return [
            {nm: np.asarray(outs[i]).reshape(NCORES, *out_avals[i].shape)[c]
             for i, nm in enumerate(out_names)}
            for c in range(NCORES)
        ]

    _RUNNER = (run, f, in_names, zero_outs)
    return _RUNNER


def _run_device(in_maps):
    run, _f, _inn, _z = _get_runner()
    return run(in_maps)


def _run_fake_device(in_maps):
    """Numpy emulation of the device program (same in/out layouts)."""
    out = []
    for im in in_maps:
        gout = np.zeros((SLOTS, P, NT, 5 * P), dtype=np.float32)
        sout = np.zeros((SLOTS, P, NT, 5 * P), dtype=np.float32)
        for s in range(SLOTS):
            acomp = im["ops"][s]                  # [P,NT,3P]
            dv = im["dvec"][:, s]           # [P,NT]
            A = np.zeros((N, N), dtype=np.float32)
            for k in range(NT):
                jlo = max(0, (k - 1) * P)
                jhi = min(N, (k + 2) * P)
                c0 = jlo - (k - 1) * P
                A[k * P:(k + 1) * P, jlo:jhi] = acomp[:, k, c0:c0 + jhi - jlo]
            M = np.eye(N, dtype=np.float32) + A @ A
            dfull = dv.T.reshape(N)
            S = dfull[:, None] * M
            G = S.T @ M
            for i in range(NT):
                jlo, jhi = max(0, i - 2), min(NT - 1, i + 2)
                sout[s, :, i, (jlo - i + 2) * P:(jhi - i + 3) * P] = \
                    S[i * P:(i + 1) * P, jlo * P:(jhi + 1) * P]
                w = min(NT - 1, i + 4) - i + 1
                gout[s, :, i, 0:w * P] = G[i * P:(i + 1) * P, i * P:(i + w) * P]
        out.append({"gout": gout, "sout": sout})
    return out


def kernel(kappa, m, H, tau, n_x=32, _fake=False):
    kappa = np.asarray(kappa, dtype=np.float32)
    m = np.asarray(m, dtype=np.float32)
    H = np.asarray(H, dtype=np.float32)
    tau = np.asarray(tau, dtype=np.float32)
    assert int(n_x) == NX
    in_maps, d = _make_in_maps(kappa, m, H, tau)
    results = _run_fake_device(in_maps) if _fake else _run_device(in_maps)
    return _assemble_Q(results, d)
